# revision 17
# baseline (speedup 1.0000x reference)
"""Cross-attention layer kernel for 8 Trainium2 NeuronCores.

Reference computation (fp32, D=1024, S=2048, B=4):
    q = x @ Wq.T + bq ; k = x @ Wk.T + bk ; v = x @ Wv.T + bv
    attn = softmax(q @ k.T / 32)
    vision = attn @ v                      # [B,S,D]
    text   = attn.T @ x                    # [B,S,D]

Sharding: core c handles batch b=c//2, sequence-half h=c%2 (1024 rows).
Those rows are both the core's queries and its share of the keys, so each
core projects Q/K/V only for its own 1024 rows.  K^T and V use per-core
key order [own half | peer half]: the own half is written straight into
SBUF by the projection evictions, a copy goes to a DRAM bounce buffer,
one 8-rank AllGather (the fast intra-chip path; 2-rank groups run ~4x
slower) exchanges all halves, and a single indirect DMA per tensor
gathers just the peer's 2MB back using a per-core index tensor
(idx = (c^1)*1024 + row) - which keeps the SPMD program identical across
cores.  Scores against the own half never wait on the exchange.  The
host unpermutes textT's per-core key order and sums pair partials.

All storage is bf16 (matmuls run 1 cycle/row, PSUM accumulates fp32).
Everything is SBUF-resident; softmax skips max-subtraction (scores are
bounded ~1.7) and the 1/rowsum normalization is folded into the outputs.

Per-core PE work: 16K x^T transposes + 3x65K projections + 131K scores
+ 16K P^T transposes + 131K vision + 131K textT ~= 620K cycles ~= 258us
at 2.4GHz.
"""

import sys

import numpy as np
import ml_dtypes

try:
    import concourse.bass as bass
except ImportError:  # pragma: no cover - grading env should have it on path
    sys.path.insert(0, "/opt/trn_rl_repo")
    import concourse.bass as bass

import concourse.mybir as mybir
import concourse.tile as tile
from concourse import bacc
from concourse.bass_utils import run_bass_kernel_spmd
from concourse.masks import make_identity

F32 = mybir.dt.float32
I32 = mybir.dt.int32
BF16 = mybir.dt.bfloat16
BF16_NP = ml_dtypes.bfloat16

B = 4          # batches
S = 2048       # sequence length
D = 1024       # model dim
SH = S // 2    # rows per core (own queries == own keys)
P = 128        # partitions
NT = D // P    # 8 tiles along d/e
NQ = SH // P   # 8 q-tiles per core
NKH = SH // P  # 8 own-half k-tiles
NK = S // P    # 16 k-tiles total
NC = S // 512  # 4 512-chunks along k (per-core order)
SCALE = 1.0 / 32.0  # 1/sqrt(D)
N512 = 512
GROUPS = [[0, 1, 2, 3, 4, 5, 6, 7]]


def build_program():
    nc = bacc.Bacc("TRN2", target_bir_lowering=False, debug=False, num_devices=8)

    xq_h = nc.dram_tensor("xq", [SH, D], BF16, kind="ExternalInput")
    wqt_h = nc.dram_tensor("wqt", [D, D], BF16, kind="ExternalInput")
    wkt_h = nc.dram_tensor("wkt", [D, D], BF16, kind="ExternalInput")
    wvt_h = nc.dram_tensor("wvt", [D, D], BF16, kind="ExternalInput")
    bq_h = nc.dram_tensor("bq", [D], F32, kind="ExternalInput")
    bk_h = nc.dram_tensor("bk", [D], F32, kind="ExternalInput")
    bv_h = nc.dram_tensor("bv", [D], F32, kind="ExternalInput")
    gidx_h = nc.dram_tensor("gidx", [P, NT], I32, kind="ExternalInput")

    vision_h = nc.dram_tensor("vision", [SH, D], BF16, kind="ExternalOutput")
    textT_h = nc.dram_tensor("textT", [D, S], BF16, kind="ExternalOutput")

    # tiled DRAM views
    xq_pid = xq_h.ap().rearrange("(i p) d -> p i d", p=P)    # [128,8,1024]
    wq_r = wqt_h.ap().rearrange("(t p) e -> p t e", p=P)     # [128,8,1024]
    wk_r = wkt_h.ap().rearrange("(t p) e -> p t e", p=P)
    wv_r = wvt_h.ap().rearrange("(t p) e -> p t e", p=P)
    bq_r = bq_h.ap().rearrange("(t p) -> p t", p=P)          # [128,8]
    bk_r = bk_h.ap().rearrange("(t p) -> p t", p=P)

    bv_ap = bv_h.ap()
    bv_bcast_src = bass.AP(tensor=bv_ap.tensor, offset=bv_ap.offset,
                           ap=[[0, P], bv_ap.ap[0]])         # [128,1024] bcast

    with tile.TileContext(nc) as tc:
        with (
            tc.tile_pool(name="singles", bufs=1) as singles,
            tc.tile_pool(name="dram", bufs=1, space="DRAM") as dram_pool,
        ):
            # DRAM bounce buffers for the 8-rank AllGathers
            kh_d = dram_pool.tile([SH, SH], BF16)       # own K^T [e, k_own]
            vh_d = dram_pool.tile([SH, D], BF16)        # own V [k_own, e]
            kg_d = dram_pool.tile([8 * SH, SH], BF16)   # gathered K^T rows
            vg_d = dram_pool.tile([8 * SH, D], BF16)    # gathered V rows

            bq_sb = singles.tile([P, NT], F32)
            nc.scalar.dma_start(out=bq_sb, in_=bq_r)
            bk_sb = singles.tile([P, NT], F32)
            nc.scalar.dma_start(out=bk_sb, in_=bk_r)
            bvb = singles.tile([P, D], F32)
            nc.scalar.dma_start(out=bvb, in_=bv_bcast_src)
            gidx = singles.tile([P, NT], I32)
            nc.scalar.dma_start(out=gidx, in_=gidx_h.ap())
            ident_f = singles.tile([P, P], F32)
            make_identity(nc, ident_f)
            ident = singles.tile([P, P], BF16)
            nc.vector.tensor_copy(ident, ident_f)
            r_all = singles.tile([P, NQ], F32)

            # whole-kernel resident tensors
            xq_sb = singles.tile([P, NQ, D], BF16)       # own rows, natural
            qt = singles.tile([P, NT, SH], BF16)         # Q^T [e, q]
            kT = singles.tile([P, NT, S], BF16)          # K^T [e, k own|peer]
            v_sb = singles.tile([P, NK, D], BF16)        # V [k own|peer, e]
            P_sb = singles.tile([P, NQ, S], BF16)        # exp(scores) [q, k]

            with (
                tc.tile_pool(name="wpool", bufs=4) as wpool,
                tc.tile_pool(name="xtpool", bufs=1) as xtpool,
                tc.tile_pool(name="trps", bufs=4, space="PSUM") as trps,
                tc.tile_pool(name="mmps", bufs=4, space="PSUM") as mmps,
            ):
                xqT = xtpool.tile([P, NT, SH], BF16, tag="xqT")  # x^T [d, q]

                def w_half(src_r, h, eng):
                    wt = wpool.tile([P, NT, N512], BF16, tag="wh", name="wt")
                    eng.dma_start(
                        out=wt, in_=src_r[:, :, h * N512:(h + 1) * N512])
                    return wt

                # weight prefetches: wk on gpsimd (t=0), wv on sync after
                # its xq half; wq reuses wk's buffers, triggered on gpsimd
                # after the K AllGather trigger (by then wk is long dead).
                wk0 = w_half(wk_r, 0, nc.gpsimd)
                wk1 = w_half(wk_r, 1, nc.gpsimd)

                # ---- phase A: load own rows, transpose into xqT ---------
                nc.sync.dma_start(out=xq_sb[:, 0:4, :], in_=xq_pid[:, 0:4, :])
                nc.scalar.dma_start(out=xq_sb[:, 4:8, :],
                                    in_=xq_pid[:, 4:8, :])
                wv0 = w_half(wv_r, 0, nc.sync)
                wv1 = w_half(wv_r, 1, nc.sync)
                for i in range(NQ):
                    for t in range(NT):
                        ps = trps.tile([P, P], BF16, tag="tr")
                        nc.tensor.transpose(
                            ps, xq_sb[:, i, t * P:(t + 1) * P], ident)
                        nc.vector.tensor_copy(
                            out=xqT[:, t, i * P:(i + 1) * P], in_=ps)

                # ---- phase B1: K^T projection -> kT own half + bounce ---
                for h, wt in ((0, wk0), (1, wk1)):
                    for tl in range(4):
                        te = h * 4 + tl
                        for kc in range(2):
                            ps = mmps.tile([P, N512], F32, tag="acc")
                            for td in range(NT):
                                nc.tensor.matmul(
                                    ps,
                                    wt[:, td, tl * P:(tl + 1) * P],
                                    xqT[:, td, kc * N512:(kc + 1) * N512],
                                    start=(td == 0), stop=(td == NT - 1))
                            nc.scalar.activation(
                                kT[:, te, kc * N512:(kc + 1) * N512], ps,
                                mybir.ActivationFunctionType.Identity,
                                bias=bk_sb[:, te:te + 1], scale=1.0)
                        eng = (nc.sync, nc.scalar)[te % 2]
                        eng.dma_start(out=kh_d[te * P:(te + 1) * P, :],
                                      in_=kT[:, te, 0:SH])
                nc.gpsimd.collective_compute(
                    "AllGather", mybir.AluOpType.bypass,
                    replica_groups=GROUPS,
                    ins=[kh_d.opt()], outs=[kg_d.opt()])
                wq0 = w_half(wq_r, 0, nc.gpsimd)
                wq1 = w_half(wq_r, 1, nc.gpsimd)
                # peer K^T half: indirect row-gathers, rows chosen by
                # the per-core index tensor (peer = core c^1)
                for t in range(NT):
                    nc.gpsimd.indirect_dma_start(
                        out=kT[:, t, SH:S],
                        out_offset=None,
                        in_=kg_d[:],
                        in_offset=bass.IndirectOffsetOnAxis(
                            ap=gidx[:, t:t + 1], axis=0),
                    )

                # ---- phase B2: V projection -> v_sb own half + bounce ---
                for ki in range(NKH):
                    for h, wt in ((0, wv0), (1, wv1)):
                        ps = mmps.tile([P, N512], F32, tag="acc")
                        for td in range(NT):
                            nc.tensor.matmul(
                                ps,
                                xqT[:, td, ki * P:(ki + 1) * P],
                                wt[:, td, :],
                                start=(td == 0), stop=(td == NT - 1))
                        nc.vector.tensor_add(
                            v_sb[:, ki, h * N512:(h + 1) * N512], ps,
                            bvb[:, h * N512:(h + 1) * N512])
                    eng = (nc.sync, nc.scalar)[ki % 2]
                    eng.dma_start(out=vh_d[ki * P:(ki + 1) * P, :],
                                  in_=v_sb[:, ki, :])
                nc.gpsimd.collective_compute(
                    "AllGather", mybir.AluOpType.bypass,
                    replica_groups=GROUPS,
                    ins=[vh_d.opt()], outs=[vg_d.opt()])
                for i in range(NKH):
                    nc.gpsimd.indirect_dma_start(
                        out=v_sb[:, NKH + i, :],
                        out_offset=None,
                        in_=vg_d[:],
                        in_offset=bass.IndirectOffsetOnAxis(
                            ap=gidx[:, i:i + 1], axis=0),
                    )

                # ---- phase B3: Q^T projection (own queries, resident) ---
                for h, wt in ((0, wq0), (1, wq1)):
                    for tl in range(4):
                        te = h * 4 + tl
                        for qc in range(2):
                            ps = mmps.tile([P, N512], F32, tag="acc")
                            for td in range(NT):
                                nc.tensor.matmul(
                                    ps,
                                    wt[:, td, tl * P:(tl + 1) * P],
                                    xqT[:, td, qc * N512:(qc + 1) * N512],
                                    start=(td == 0), stop=(td == NT - 1))
                            nc.scalar.activation(
                                qt[:, te, qc * N512:(qc + 1) * N512], ps,
                                mybir.ActivationFunctionType.Identity,
                                bias=bq_sb[:, te:te + 1], scale=1.0)

                # ---- phase C1: scores + exp(+rowsum); P resident --------
                with tc.tile_pool(name="phC1_l", bufs=4) as phC1_l:
                    for j in range(NQ):
                        l4 = phC1_l.tile([P, NC], F32, tag="l4")
                        for kc in range(NC):
                            ps = mmps.tile([P, N512], F32, tag="acc")
                            for t in range(NT):
                                nc.tensor.matmul(
                                    ps,
                                    qt[:, t, j * P:(j + 1) * P],
                                    kT[:, t, kc * N512:(kc + 1) * N512],
                                    start=(t == 0), stop=(t == NT - 1))
                            nc.scalar.activation(
                                P_sb[:, j, kc * N512:(kc + 1) * N512], ps,
                                mybir.ActivationFunctionType.Exp,
                                bias=0.0, scale=SCALE,
                                accum_out=l4[:, kc:kc + 1])
                        lsum = phC1_l.tile([P, 1], F32, tag="lsum")
                        nc.vector.reduce_sum(out=lsum, in_=l4,
                                             axis=mybir.AxisListType.X)
                        nc.vector.reciprocal(out=r_all[:, j:j + 1], in_=lsum)

            # ---- phase C2: P^T transposes + vision (pipelined) ----------
            with (
                tc.tile_pool(name="phD_xs", bufs=1) as phD_xs,
                tc.tile_pool(name="phC2_pt", bufs=2) as phC2_pt,
                tc.tile_pool(name="phC2_ev", bufs=3) as phC2_ev,
            ):
                # scale phase D's x_q while C2 computes
                xs = phD_xs.tile([P, NQ, D], BF16, tag="xs")
                for j in range(NQ):
                    nc.vector.tensor_scalar_mul(
                        xs[:, j, :], xq_sb[:, j, :], r_all[:, j:j + 1])

                with (
                    tc.tile_pool(name="phC2_tr", bufs=2,
                                 space="PSUM") as phC2_tr,
                    tc.tile_pool(name="phC2_vp", bufs=4,
                                 space="PSUM") as phC2_vp,
                ):
                    def transposes(j):
                        ptj = phC2_pt.tile([P, NK, P], BF16, tag="ptj",
                                           name="ptj")
                        for i in range(NK):
                            ps = phC2_tr.tile([P, P], BF16, tag="tr")
                            nc.tensor.transpose(
                                ps, P_sb[:, j, i * P:(i + 1) * P], ident)
                            nc.vector.tensor_copy(out=ptj[:, i, :], in_=ps)
                        return ptj

                    def vision(j, ptj):
                        ev = phC2_ev.tile([P, D], BF16, tag="ev")
                        for h in range(2):
                            ps = phC2_vp.tile([P, N512], F32, tag="vp")
                            for i in range(NK):
                                nc.tensor.matmul(
                                    ps,
                                    ptj[:, i, :],
                                    v_sb[:, i, h * N512:(h + 1) * N512],
                                    start=(i == 0), stop=(i == NK - 1))
                            nc.scalar.activation(
                                ev[:, h * N512:(h + 1) * N512], ps,
                                mybir.ActivationFunctionType.Copy,
                                bias=0.0, scale=r_all[:, j:j + 1])
                        eng = (nc.sync, nc.scalar, nc.gpsimd)[j % 3]
                        eng.dma_start(
                            out=vision_h.ap()[j * P:(j + 1) * P, :], in_=ev)

                    prev = transposes(0)
                    for j in range(1, NQ):
                        cur = transposes(j)
                        vision(j - 1, prev)
                        prev = cur
                    vision(NQ - 1, prev)

                # ---- phase D: textT = (x_q * r).T @ P -------------------
                with (
                    tc.tile_pool(name="phD_ev", bufs=3) as phD_ev,
                    tc.tile_pool(name="phD_ps", bufs=8, space="PSUM") as phD_ps,
                ):
                    for dc in range(NT):
                        ev = phD_ev.tile([P, S], BF16, tag="ev")
                        for kc in range(NC):
                            ps = phD_ps.tile([P, N512], F32, tag="tp")
                            for j in range(NQ):
                                nc.tensor.matmul(
                                    ps,
                                    xs[:, j, dc * P:(dc + 1) * P],
                                    P_sb[:, j, kc * N512:(kc + 1) * N512],
                                    start=(j == 0), stop=(j == NQ - 1))
                            nc.scalar.copy(
                                out=ev[:, kc * N512:(kc + 1) * N512], in_=ps)
                        eng = (nc.sync, nc.scalar, nc.gpsimd)[dc % 3]
                        eng.dma_start(
                            out=textT_h.ap()[dc * P:(dc + 1) * P, :], in_=ev)

    nc.compile()
    return nc


_NC_CACHE = []


def _get_program():
    if not _NC_CACHE:
        _NC_CACHE.append(build_program())
    return _NC_CACHE[0]


def kernel(inputs, Wq, bq, Wk, bk, Wv, bv, _run_opts=None):
    x = np.asarray(inputs, dtype=np.float32).astype(BF16_NP)
    WqT = np.ascontiguousarray(np.asarray(Wq, dtype=np.float32).T).astype(BF16_NP)
    WkT = np.ascontiguousarray(np.asarray(Wk, dtype=np.float32).T).astype(BF16_NP)
    WvT = np.ascontiguousarray(np.asarray(Wv, dtype=np.float32).T).astype(BF16_NP)
    bq = np.ascontiguousarray(np.asarray(bq, dtype=np.float32))
    bk = np.ascontiguousarray(np.asarray(bk, dtype=np.float32))
    bv = np.ascontiguousarray(np.asarray(bv, dtype=np.float32))

    nc = _get_program()

    p_idx = np.arange(P, dtype=np.int32)
    t_idx = np.arange(NT, dtype=np.int32)
    in_maps = []
    for c in range(8):
        b, h = divmod(c, 2)
        xq = np.ascontiguousarray(x[b, h * SH:(h + 1) * SH])
        gidx = ((c ^ 1) * SH + t_idx[None, :] * P + p_idx[:, None]).astype(
            np.int32)
        in_maps.append({
            "xq": xq, "gidx": gidx,
            "wqt": WqT, "wkt": WkT, "wvt": WvT,
            "bq": bq, "bk": bk, "bv": bv,
        })

    run_opts = dict(_run_opts or {})
    res = run_bass_kernel_spmd(nc, in_maps, core_ids=list(range(8)), **run_opts)
    results = res.results

    vision = np.empty((B, S, D), np.float32)
    text = np.zeros((B, S, D), np.float32)
    for c in range(8):
        b, h = divmod(c, 2)
        vision[b, h * SH:(h + 1) * SH] = results[c]["vision"].astype(np.float32)
        tT = results[c]["textT"].astype(np.float32)  # k order [own | peer]
        text[b, h * SH:(h + 1) * SH] += tT[:, :SH].T
        text[b, (1 - h) * SH:(2 - h) * SH] += tT[:, SH:].T
    if _run_opts is not None:
        return (vision, text), res
    return (vision, text)


# revision 19
# speedup vs baseline: 1.6258x; 1.6258x over previous
"""Cross-attention layer kernel for 8 Trainium2 NeuronCores.

Reference computation (fp32, D=1024, S=2048, B=4):
    q = x @ Wq.T + bq ; k = x @ Wk.T + bk ; v = x @ Wv.T + bv
    attn = softmax(q @ k.T / 32)
    vision = attn @ v                      # [B,S,D]
    text   = attn.T @ x                    # [B,S,D]

Sharding: core c handles batch b=c//2, sequence-half h=c%2 (1024 rows).
Those rows are both the core's queries and its share of the keys, so each
core projects Q/K/V only for its own 1024 rows.  K^T and V use per-core
key order [own half | peer half]: the own half is written straight into
SBUF by the projection evictions, a copy goes to a DRAM bounce buffer,
one 8-rank AllGather (the fast intra-chip path; 2-rank groups run ~4x
slower) exchanges all halves, and a single indirect DMA per tensor
gathers just the peer's 2MB back using a per-core index tensor
(idx = (c^1)*1024 + row) - which keeps the SPMD program identical across
cores.  Scores against the own half never wait on the exchange.  The
host unpermutes textT's per-core key order and sums pair partials.

All storage is bf16 (matmuls run 1 cycle/row, PSUM accumulates fp32).
Everything is SBUF-resident; softmax skips max-subtraction (scores are
bounded ~1.7) and the 1/rowsum normalization is folded into the outputs.

Per-core PE work: 16K x^T transposes + 3x65K projections + 131K scores
+ 16K P^T transposes + 131K vision + 131K textT ~= 620K cycles ~= 258us
at 2.4GHz.
"""

import sys

import numpy as np
import ml_dtypes

try:
    import concourse.bass as bass
except ImportError:  # pragma: no cover - grading env should have it on path
    sys.path.insert(0, "/opt/trn_rl_repo")
    import concourse.bass as bass

import concourse.mybir as mybir
import concourse.tile as tile
from concourse import bacc
from concourse.bass_utils import run_bass_kernel_spmd
from concourse.masks import make_identity

F32 = mybir.dt.float32
I32 = mybir.dt.int32
BF16 = mybir.dt.bfloat16
BF16_NP = ml_dtypes.bfloat16

B = 4          # batches
S = 2048       # sequence length
D = 1024       # model dim
SH = S // 2    # rows per core (own queries == own keys)
P = 128        # partitions
NT = D // P    # 8 tiles along d/e
NQ = SH // P   # 8 q-tiles per core
NKH = SH // P  # 8 own-half k-tiles
NK = S // P    # 16 k-tiles total
NC = S // 512  # 4 512-chunks along k (per-core order)
SCALE = 1.0 / 32.0  # 1/sqrt(D)
N512 = 512
GROUPS = [[0, 1], [2, 3], [4, 5], [6, 7]]


def build_program():
    nc = bacc.Bacc("TRN2", target_bir_lowering=False, debug=False, num_devices=8)

    xq_h = nc.dram_tensor("xq", [SH, D], BF16, kind="ExternalInput")
    wqt_h = nc.dram_tensor("wqt", [D, D], BF16, kind="ExternalInput")
    wkt_h = nc.dram_tensor("wkt", [D, D], BF16, kind="ExternalInput")
    wvt_h = nc.dram_tensor("wvt", [D, D], BF16, kind="ExternalInput")
    bq_h = nc.dram_tensor("bq", [D], F32, kind="ExternalInput")
    bk_h = nc.dram_tensor("bk", [D], F32, kind="ExternalInput")
    bv_h = nc.dram_tensor("bv", [D], F32, kind="ExternalInput")
    gidx_h = nc.dram_tensor("gidx", [P, NT], I32, kind="ExternalInput")

    vision_h = nc.dram_tensor("vision", [SH, D], BF16, kind="ExternalOutput")
    textT_h = nc.dram_tensor("textT", [D, S], BF16, kind="ExternalOutput")

    # tiled DRAM views
    xq_pid = xq_h.ap().rearrange("(i p) d -> p i d", p=P)    # [128,8,1024]
    wq_r = wqt_h.ap().rearrange("(t p) e -> p t e", p=P)     # [128,8,1024]
    wk_r = wkt_h.ap().rearrange("(t p) e -> p t e", p=P)
    wv_r = wvt_h.ap().rearrange("(t p) e -> p t e", p=P)
    bq_r = bq_h.ap().rearrange("(t p) -> p t", p=P)          # [128,8]
    bk_r = bk_h.ap().rearrange("(t p) -> p t", p=P)

    bv_ap = bv_h.ap()
    bv_bcast_src = bass.AP(tensor=bv_ap.tensor, offset=bv_ap.offset,
                           ap=[[0, P], bv_ap.ap[0]])         # [128,1024] bcast

    with tile.TileContext(nc) as tc:
        with (
            tc.tile_pool(name="singles", bufs=1) as singles,
            tc.tile_pool(name="dram", bufs=1, space="DRAM") as dram_pool,
        ):
            # DRAM bounce buffers for the 8-rank AllGathers
            kh_d = dram_pool.tile([SH, SH], BF16)       # own K^T [e, k_own]
            vh_d = dram_pool.tile([SH, D], BF16)        # own V [k_own, e]
            kg_d = dram_pool.tile([2 * SH, SH], BF16)   # gathered K^T rows
            vg_d = dram_pool.tile([2 * SH, D], BF16)    # gathered V rows

            bq_sb = singles.tile([P, NT], F32)
            nc.scalar.dma_start(out=bq_sb, in_=bq_r)
            bk_sb = singles.tile([P, NT], F32)
            nc.scalar.dma_start(out=bk_sb, in_=bk_r)
            bvb = singles.tile([P, D], F32)
            nc.scalar.dma_start(out=bvb, in_=bv_bcast_src)
            gidx = singles.tile([P, NT], I32)
            nc.scalar.dma_start(out=gidx, in_=gidx_h.ap())
            ident_f = singles.tile([P, P], F32)
            make_identity(nc, ident_f)
            ident = singles.tile([P, P], BF16)
            nc.vector.tensor_copy(ident, ident_f)
            r_all = singles.tile([P, NQ], F32)

            # whole-kernel resident tensors
            xq_sb = singles.tile([P, NQ, D], BF16)       # own rows, natural
            qt = singles.tile([P, NT, SH], BF16)         # Q^T [e, q]
            kT = singles.tile([P, NT, S], BF16)          # K^T [e, k own|peer]
            v_sb = singles.tile([P, NK, D], BF16)        # V [k own|peer, e]
            P_sb = singles.tile([P, NQ, S], BF16)        # exp(scores) [q, k]

            with (
                tc.tile_pool(name="wpool", bufs=4) as wpool,
                tc.tile_pool(name="xtpool", bufs=1) as xtpool,
                tc.tile_pool(name="trps", bufs=4, space="PSUM") as trps,
                tc.tile_pool(name="mmps", bufs=4, space="PSUM") as mmps,
            ):
                xqT = xtpool.tile([P, NT, SH], BF16, tag="xqT")  # x^T [d, q]

                def w_half(src_r, h, eng):
                    wt = wpool.tile([P, NT, N512], BF16, tag="wh", name="wt")
                    eng.dma_start(
                        out=wt, in_=src_r[:, :, h * N512:(h + 1) * N512])
                    return wt

                # weight prefetches: wk on gpsimd (t=0), wv on sync after
                # its xq half; wq reuses wk's buffers, triggered on gpsimd
                # after the K AllGather trigger (by then wk is long dead).
                wk0 = w_half(wk_r, 0, nc.gpsimd)
                wk1 = w_half(wk_r, 1, nc.gpsimd)

                # ---- phase A: load own rows, transpose into xqT ---------
                nc.sync.dma_start(out=xq_sb[:, 0:4, :], in_=xq_pid[:, 0:4, :])
                nc.scalar.dma_start(out=xq_sb[:, 4:8, :],
                                    in_=xq_pid[:, 4:8, :])
                wv0 = w_half(wv_r, 0, nc.sync)
                wv1 = w_half(wv_r, 1, nc.sync)
                for i in range(NQ):
                    for t in range(NT):
                        ps = trps.tile([P, P], BF16, tag="tr")
                        nc.tensor.transpose(
                            ps, xq_sb[:, i, t * P:(t + 1) * P], ident)
                        nc.vector.tensor_copy(
                            out=xqT[:, t, i * P:(i + 1) * P], in_=ps)

                # ---- phase B1: K^T projection -> kT own half + bounce ---
                for h, wt in ((0, wk0), (1, wk1)):
                    for tl in range(4):
                        te = h * 4 + tl
                        for kc in range(2):
                            ps = mmps.tile([P, N512], F32, tag="acc")
                            for td in range(NT):
                                nc.tensor.matmul(
                                    ps,
                                    wt[:, td, tl * P:(tl + 1) * P],
                                    xqT[:, td, kc * N512:(kc + 1) * N512],
                                    start=(td == 0), stop=(td == NT - 1))
                            nc.scalar.activation(
                                kT[:, te, kc * N512:(kc + 1) * N512], ps,
                                mybir.ActivationFunctionType.Identity,
                                bias=bk_sb[:, te:te + 1], scale=1.0)
                        eng = (nc.sync, nc.scalar)[te % 2]
                        eng.dma_start(out=kh_d[te * P:(te + 1) * P, :],
                                      in_=kT[:, te, 0:SH])
                nc.gpsimd.collective_compute(
                    "AllGather", mybir.AluOpType.bypass,
                    replica_groups=GROUPS,
                    ins=[kh_d.opt()], outs=[kg_d.opt()])
                wq0 = w_half(wq_r, 0, nc.gpsimd)
                wq1 = w_half(wq_r, 1, nc.gpsimd)
                # peer K^T half: indirect row-gathers, rows chosen by
                # the per-core index tensor (peer = core c^1)
                for t in range(NT):
                    nc.gpsimd.indirect_dma_start(
                        out=kT[:, t, SH:S],
                        out_offset=None,
                        in_=kg_d[:],
                        in_offset=bass.IndirectOffsetOnAxis(
                            ap=gidx[:, t:t + 1], axis=0),
                    )

                # ---- phase B2: V projection -> v_sb own half + bounce ---
                for ki in range(NKH):
                    for h, wt in ((0, wv0), (1, wv1)):
                        ps = mmps.tile([P, N512], F32, tag="acc")
                        for td in range(NT):
                            nc.tensor.matmul(
                                ps,
                                xqT[:, td, ki * P:(ki + 1) * P],
                                wt[:, td, :],
                                start=(td == 0), stop=(td == NT - 1))
                        nc.vector.tensor_add(
                            v_sb[:, ki, h * N512:(h + 1) * N512], ps,
                            bvb[:, h * N512:(h + 1) * N512])
                    eng = (nc.sync, nc.scalar)[ki % 2]
                    eng.dma_start(out=vh_d[ki * P:(ki + 1) * P, :],
                                  in_=v_sb[:, ki, :])
                nc.gpsimd.collective_compute(
                    "AllGather", mybir.AluOpType.bypass,
                    replica_groups=GROUPS,
                    ins=[vh_d.opt()], outs=[vg_d.opt()])
                for i in range(NKH):
                    nc.gpsimd.indirect_dma_start(
                        out=v_sb[:, NKH + i, :],
                        out_offset=None,
                        in_=vg_d[:],
                        in_offset=bass.IndirectOffsetOnAxis(
                            ap=gidx[:, i:i + 1], axis=0),
                    )

                # ---- phase B3: Q^T projection (own queries, resident) ---
                for h, wt in ((0, wq0), (1, wq1)):
                    for tl in range(4):
                        te = h * 4 + tl
                        for qc in range(2):
                            ps = mmps.tile([P, N512], F32, tag="acc")
                            for td in range(NT):
                                nc.tensor.matmul(
                                    ps,
                                    wt[:, td, tl * P:(tl + 1) * P],
                                    xqT[:, td, qc * N512:(qc + 1) * N512],
                                    start=(td == 0), stop=(td == NT - 1))
                            nc.scalar.activation(
                                qt[:, te, qc * N512:(qc + 1) * N512], ps,
                                mybir.ActivationFunctionType.Identity,
                                bias=bq_sb[:, te:te + 1], scale=1.0)

                # ---- phase C1: scores + exp(+rowsum); P resident --------
                # kc-major: own-half scores (kc 0-1) run first across all
                # q-tiles so the peer K^T gather has ~27us more to land.
                with tc.tile_pool(name="phC1_l", bufs=1) as phC1_l:
                    l4 = phC1_l.tile([P, NQ, NC], F32, tag="l4")
                    for kc in range(NC):
                        for j in range(NQ):
                            ps = mmps.tile([P, N512], F32, tag="acc")
                            for t in range(NT):
                                nc.tensor.matmul(
                                    ps,
                                    qt[:, t, j * P:(j + 1) * P],
                                    kT[:, t, kc * N512:(kc + 1) * N512],
                                    start=(t == 0), stop=(t == NT - 1))
                            nc.scalar.activation(
                                P_sb[:, j, kc * N512:(kc + 1) * N512], ps,
                                mybir.ActivationFunctionType.Exp,
                                bias=0.0, scale=SCALE,
                                accum_out=l4[:, j, kc:kc + 1])
                    for j in range(NQ):
                        lsum = phC1_l.tile([P, 1], F32, tag="lsum", bufs=4)
                        nc.vector.reduce_sum(out=lsum, in_=l4[:, j, :],
                                             axis=mybir.AxisListType.X)
                        nc.vector.reciprocal(out=r_all[:, j:j + 1], in_=lsum)

            # ---- phase D then C2: textT first (needs no V), so the
            # peer-V gather has until the vision phase to land ------------
            with (
                tc.tile_pool(name="phD_xs", bufs=1) as phD_xs,
                tc.tile_pool(name="phC2_pt", bufs=2) as phC2_pt,
                tc.tile_pool(name="phC2_ev", bufs=3) as phC2_ev,
            ):
                # scale phase D's x_q
                xs = phD_xs.tile([P, NQ, D], BF16, tag="xs")
                for j in range(NQ):
                    nc.vector.tensor_scalar_mul(
                        xs[:, j, :], xq_sb[:, j, :], r_all[:, j:j + 1])

                # ---- phase D: textT = (x_q * r).T @ P -------------------
                with (
                    tc.tile_pool(name="phD_ev", bufs=3) as phD_ev,
                    tc.tile_pool(name="phD_ps", bufs=8, space="PSUM") as phD_ps,
                ):
                    for dc in range(NT):
                        ev = phD_ev.tile([P, S], BF16, tag="ev")
                        for kc in range(NC):
                            ps = phD_ps.tile([P, N512], F32, tag="tp")
                            for j in range(NQ):
                                nc.tensor.matmul(
                                    ps,
                                    xs[:, j, dc * P:(dc + 1) * P],
                                    P_sb[:, j, kc * N512:(kc + 1) * N512],
                                    start=(j == 0), stop=(j == NQ - 1))
                            nc.scalar.copy(
                                out=ev[:, kc * N512:(kc + 1) * N512], in_=ps)
                        eng = (nc.sync, nc.scalar, nc.gpsimd)[dc % 3]
                        eng.dma_start(
                            out=textT_h.ap()[dc * P:(dc + 1) * P, :], in_=ev)

                # ---- phase C2: P^T transposes + vision (pipelined) ------
                with (
                    tc.tile_pool(name="phC2_tr", bufs=2,
                                 space="PSUM") as phC2_tr,
                    tc.tile_pool(name="phC2_vp", bufs=4,
                                 space="PSUM") as phC2_vp,
                ):
                    def transposes(j):
                        ptj = phC2_pt.tile([P, NK, P], BF16, tag="ptj",
                                           name="ptj")
                        for i in range(NK):
                            ps = phC2_tr.tile([P, P], BF16, tag="tr")
                            nc.tensor.transpose(
                                ps, P_sb[:, j, i * P:(i + 1) * P], ident)
                            nc.vector.tensor_copy(out=ptj[:, i, :], in_=ps)
                        return ptj

                    def vision(j, ptj):
                        ev = phC2_ev.tile([P, D], BF16, tag="ev")
                        for h in range(2):
                            ps = phC2_vp.tile([P, N512], F32, tag="vp")
                            for i in range(NK):
                                nc.tensor.matmul(
                                    ps,
                                    ptj[:, i, :],
                                    v_sb[:, i, h * N512:(h + 1) * N512],
                                    start=(i == 0), stop=(i == NK - 1))
                            nc.scalar.activation(
                                ev[:, h * N512:(h + 1) * N512], ps,
                                mybir.ActivationFunctionType.Copy,
                                bias=0.0, scale=r_all[:, j:j + 1])
                        eng = (nc.sync, nc.scalar, nc.gpsimd)[j % 3]
                        eng.dma_start(
                            out=vision_h.ap()[j * P:(j + 1) * P, :], in_=ev)

                    prev = transposes(0)
                    for j in range(1, NQ):
                        cur = transposes(j)
                        vision(j - 1, prev)
                        prev = cur
                    vision(NQ - 1, prev)

    nc.compile()
    return nc


_NC_CACHE = []


def _get_program():
    if not _NC_CACHE:
        _NC_CACHE.append(build_program())
    return _NC_CACHE[0]


def kernel(inputs, Wq, bq, Wk, bk, Wv, bv, _run_opts=None):
    x = np.asarray(inputs, dtype=np.float32).astype(BF16_NP)
    WqT = np.ascontiguousarray(np.asarray(Wq, dtype=np.float32).T).astype(BF16_NP)
    WkT = np.ascontiguousarray(np.asarray(Wk, dtype=np.float32).T).astype(BF16_NP)
    WvT = np.ascontiguousarray(np.asarray(Wv, dtype=np.float32).T).astype(BF16_NP)
    bq = np.ascontiguousarray(np.asarray(bq, dtype=np.float32))
    bk = np.ascontiguousarray(np.asarray(bk, dtype=np.float32))
    bv = np.ascontiguousarray(np.asarray(bv, dtype=np.float32))

    nc = _get_program()

    p_idx = np.arange(P, dtype=np.int32)
    t_idx = np.arange(NT, dtype=np.int32)
    in_maps = []
    for c in range(8):
        b, h = divmod(c, 2)
        xq = np.ascontiguousarray(x[b, h * SH:(h + 1) * SH])
        gidx = ((1 - h) * SH + t_idx[None, :] * P + p_idx[:, None]).astype(
            np.int32)
        in_maps.append({
            "xq": xq, "gidx": gidx,
            "wqt": WqT, "wkt": WkT, "wvt": WvT,
            "bq": bq, "bk": bk, "bv": bv,
        })

    run_opts = dict(_run_opts or {})
    res = run_bass_kernel_spmd(nc, in_maps, core_ids=list(range(8)), **run_opts)
    results = res.results

    vision = np.empty((B, S, D), np.float32)
    text = np.zeros((B, S, D), np.float32)
    for c in range(8):
        b, h = divmod(c, 2)
        vision[b, h * SH:(h + 1) * SH] = results[c]["vision"].astype(np.float32)
        tT = results[c]["textT"].astype(np.float32)  # k order [own | peer]
        text[b, h * SH:(h + 1) * SH] += tT[:, :SH].T
        text[b, (1 - h) * SH:(2 - h) * SH] += tT[:, SH:].T
    if _run_opts is not None:
        return (vision, text), res
    return (vision, text)
